# revision 16
# baseline (speedup 1.0000x reference)
"""Trainium2 Bass kernel: VQ-codebook soft assignment (ClusteringLayer).

q[n, k] = t_nk / sum_j t_nj,  t_nk = 1 / (1 + ||x_n - c_k||^2)
(reference has ALPHA = 1.0, so the power (ALPHA+1)/2 == 1.0 is the identity)

Strategy (data-parallel over 8 cores, rows sharded, centroids replicated):
 - host: pad N to 8*63488, precompute per-row x_sq, and the augmented
   centroid matrix W[66, 100] = [-2*C^T ; ones-coeff rows] so that ONE
   matmul per 128-row chunk produces a = 1 + ||x-c||^2 directly.
 - device, per 1024-row macro-tile:
     DMA in (rows packed 2/partition -> 512B contiguous reads)
     PE   : transpose [128, 66] chunks -> X^T (with x_sq + ones rows)
     PE   : matmul  X^T_aug (lhsT) @ W -> PSUM a = 1 + d2   [128, 100] x8
     DVE  : reciprocal_approx_fast (PSUM->SBUF)  t = 1/a    [128, 8, 100]
     DVE  : tensor_reduce row-sums s [128, 8]
     GPSIMD/ACT : normalize  q = t / s
     DMA out (800B contiguous writes)
"""

import os
from contextlib import ExitStack

import numpy as np

try:  # persistent XLA compile cache: makes repeat runs skip the walrus compile
    import jax

    jax.config.update("jax_compilation_cache_dir", "/tmp/jax_comp_cache")
    jax.config.update("jax_persistent_cache_min_entry_size_bytes", -1)
    jax.config.update("jax_persistent_cache_min_compile_time_secs", 0.0)
except Exception:
    pass

import concourse.bacc as bacc
import concourse.bass as bass
import concourse.mybir as mybir
from concourse.bass_utils import run_bass_kernel_spmd
from concourse.tile import TileContext

# problem constants (hardcoded; kernel.py must be self-contained)
N = 500_000
D = 64
K = 100
N_CORES = 8

P = 128                  # partitions; rows per chunk
RJ = 2                   # rows packed per partition
G = 4                    # row groups per macro-tile
CHUNKS = G * RJ          # 8 chunks per macro
MACRO_ROWS = P * RJ * G  # 1024
CDIM = D + 2             # features + x_sq col + ones col
N_MACROS = 62
ROWS_PC = N_MACROS * MACRO_ROWS  # 63488 rows per core
N_PAD = ROWS_PC * N_CORES        # 507904

F32 = mybir.dt.float32

# how many of the 8 per-macro normalize ops run on ACT instead of GPSIMD
ACT_CHUNKS = 2


def build_program(
    n_macros: int = N_MACROS, act_chunks: int = ACT_CHUNKS, passes: int = 1
) -> bass.Bass:
    rows = n_macros * MACRO_ROWS
    nc = bacc.Bacc()
    # x is pre-augmented on the host: 64 features + x_sq col + ones col
    x = nc.declare_dram_parameter("x", [rows, CDIM], F32, isOutput=False)
    w = nc.declare_dram_parameter("w", [CDIM, K], F32, isOutput=False)
    q = nc.declare_dram_parameter("q", [rows, K], F32, isOutput=True)
    ident = nc.inline_tensor(np.eye(P, dtype=np.float32), name="ident")

    # row(m, g, p, j) = m*1024 + g*256 + 2p + j
    x_v = x[:, :].rearrange("(m g p j) c -> m p g (j c)", g=G, p=P, j=RJ)
    q_v = q[:, :].rearrange("(m g p j) k -> m p g j k", g=G, p=P, j=RJ)

    with TileContext(nc) as tc, ExitStack() as ctx:
        consts = ctx.enter_context(tc.tile_pool(name="consts", bufs=1))
        w_t = consts.tile([CDIM, K], F32)
        nc.sync.dma_start(out=w_t[:, :], in_=w[:, :])
        id_t = consts.tile([P, P], F32)
        nc.sync.dma_start(out=id_t[:, :], in_=ident[:, :])

        xe_pool = ctx.enter_context(tc.tile_pool(name="xe", bufs=3))
        pt_pool = ctx.enter_context(tc.tile_pool(name="pt", bufs=4, space="PSUM"))
        xt_pool = ctx.enter_context(tc.tile_pool(name="xt", bufs=4))
        pa_pool = ctx.enter_context(tc.tile_pool(name="pa", bufs=2, space="PSUM"))
        qun_pool = ctx.enter_context(tc.tile_pool(name="qun", bufs=2))
        s_pool = ctx.enter_context(tc.tile_pool(name="s", bufs=2))
        r_pool = ctx.enter_context(tc.tile_pool(name="r", bufs=2))
        qt_pool = ctx.enter_context(tc.tile_pool(name="qt", bufs=3))

        def macro_body():
            for m in range(n_macros):
                emit_macro(m)

        def emit_macro(m):
            xe = xe_pool.tile([P, G * RJ * CDIM], F32)
            xe_v = xe[:, :].rearrange("p (g j c) -> p g j c", g=G, j=RJ)
            nc.sync.dma_start(out=xe[:, :], in_=x_v[m])

            xts = []
            for h in range(2):
                pt = pt_pool.tile([CDIM, 4 * P], F32)
                for c4 in range(4):
                    g = h * 2 + c4 // 2
                    j = c4 % 2
                    nc.tensor.transpose(
                        pt[:, c4 * P : (c4 + 1) * P], xe_v[:, g, j, :], id_t[:, :]
                    )
                xt = xt_pool.tile([CDIM, 4 * P], F32)
                nc.scalar.copy(xt[:, :], pt[:, :])
                xts.append(xt)

            pa = pa_pool.tile([P, CHUNKS * P], F32)  # [128, 1024] = 2 banks
            for c in range(CHUNKS):
                nc.tensor.matmul(
                    pa[:, c * P : c * P + K],
                    xts[c // 4][:, (c % 4) * P : (c % 4 + 1) * P],
                    w_t[:, :],
                    start=True,
                    stop=True,
                )

            qun = qun_pool.tile([P, CHUNKS * K], F32)
            pa_v = pa[:, :].rearrange("p (c s) -> p c s", c=CHUNKS)
            qun_v = qun[:, :].rearrange("p (c k) -> p c k", c=CHUNKS)
            nc.vector.reciprocal_approx_fast(out=qun_v, in_=pa_v[:, :, 0:K])

            s_t = s_pool.tile([P, CHUNKS], F32)
            nc.vector.tensor_reduce(
                s_t[:, :], qun_v, axis=mybir.AxisListType.X, op=mybir.AluOpType.add
            )

            qt = qt_pool.tile([P, CHUNKS * K], F32)
            n_gp = CHUNKS - act_chunks
            if act_chunks:
                r_t = r_pool.tile([P, act_chunks], F32)
                nc.vector.reciprocal_approx_fast(
                    out=r_t[:, :], in_=s_t[:, n_gp:CHUNKS]
                )
            for c in range(n_gp):
                nc.gpsimd.normalize_recip(
                    qt[:, c * K : (c + 1) * K],
                    qun[:, c * K : (c + 1) * K],
                    s_t[:, c : c + 1],
                )
            for i in range(act_chunks):
                c = n_gp + i
                nc.scalar.activation(
                    qt[:, c * K : (c + 1) * K],
                    qun[:, c * K : (c + 1) * K],
                    mybir.ActivationFunctionType.Copy,
                    scale=r_t[:, i : i + 1],
                )

            qt_v = qt[:, :].rearrange("p (g j k) -> p g j k", g=G, j=RJ)
            nc.sync.dma_start(out=q_v[m], in_=qt_v)

        if passes > 1:
            with tc.For_i(0, passes, 1):
                macro_body()
        else:
            macro_body()

    nc.compile()
    return nc


def host_prep(x: np.ndarray, clusters: np.ndarray, rows_total: int):
    """Pad + build the augmented input [rows, 66] and centroid matrix."""
    n = x.shape[0]
    xall = np.zeros((rows_total, CDIM), dtype=np.float32)
    xall[:n, :D] = x
    xall[:n, D] = np.einsum("nd,nd->n", x, x)
    xall[:, D + 1] = 1.0
    c = np.asarray(clusters, dtype=np.float32)
    c_sq = np.einsum("kd,kd->k", c, c)
    w = np.empty((CDIM, K), dtype=np.float32)
    w[:D] = -2.0 * c.T
    w[D] = 1.0             # coefficient of the x_sq row
    w[D + 1] = 1.0 + c_sq  # coefficient of the ones row
    return xall, w


_CACHE: dict = {}

LAST_RESULT = None  # BassKernelResults of the most recent kernel() call


def kernel(inputs: np.ndarray, clusters: np.ndarray) -> np.ndarray:
    global LAST_RESULT
    x = np.ascontiguousarray(np.asarray(inputs, dtype=np.float32))
    n = x.shape[0]
    assert n == N and x.shape[1] == D, f"unexpected input shape {x.shape}"

    xall, w = host_prep(x, clusters, N_PAD)

    if "nc" not in _CACHE:
        _CACHE["nc"] = build_program()
    nc = _CACHE["nc"]

    in_maps = []
    for i in range(N_CORES):
        sl = slice(i * ROWS_PC, (i + 1) * ROWS_PC)
        in_maps.append({"x": np.ascontiguousarray(xall[sl]), "w": w})

    res = run_bass_kernel_spmd(nc, in_maps, list(range(N_CORES)))
    LAST_RESULT = res
    out = np.concatenate([res.results[i]["q"] for i in range(N_CORES)], axis=0)
    return np.ascontiguousarray(out[:n])


# revision 27
# speedup vs baseline: 1.3142x; 1.3142x over previous
"""Trainium2 Bass kernel: VQ-codebook soft assignment (ClusteringLayer).

q[n, k] = t_nk / sum_j t_nj,  t_nk = 1 / (1 + ||x_n - c_k||^2)
(reference has ALPHA = 1.0, so the power (ALPHA+1)/2 == 1.0 is the identity)

Strategy (data-parallel over 8 cores, rows sharded, centroids replicated):
 - host: pad N to 8*63488, precompute per-row x_sq, and the augmented
   centroid matrix W[66, 100] = [-2*C^T ; ones-coeff rows] so that ONE
   matmul per 128-row chunk produces a = 1 + ||x-c||^2 directly.
 - device, per 1024-row macro-tile:
     DMA in (rows packed 2/partition -> 512B contiguous reads)
     PE   : transpose [128, 66] chunks -> X^T (with x_sq + ones rows)
     PE   : matmul  X^T_aug (lhsT) @ W -> PSUM a = 1 + d2   [128, 100] x8
     DVE  : reciprocal_approx_fast (PSUM->SBUF)  t = 1/a    [128, 8, 100]
     DVE  : tensor_reduce row-sums s [128, 8]
     GPSIMD/ACT : normalize  q = t / s
     DMA out (800B contiguous writes)
"""

import os
from contextlib import ExitStack

import numpy as np

try:  # persistent XLA compile cache: makes repeat runs skip the walrus compile
    import jax

    jax.config.update("jax_compilation_cache_dir", "/tmp/jax_comp_cache")
    jax.config.update("jax_persistent_cache_min_entry_size_bytes", -1)
    jax.config.update("jax_persistent_cache_min_compile_time_secs", 0.0)
except Exception:
    pass

import concourse.bacc as bacc
import concourse.bass as bass
import concourse.mybir as mybir
from concourse.bass_utils import run_bass_kernel_spmd
from concourse.tile import TileContext

# problem constants (hardcoded; kernel.py must be self-contained)
N = 500_000
D = 64
K = 100
N_CORES = 8

P = 128                  # partitions; rows per chunk
RJ = 2                   # rows packed per partition
G = 4                    # row groups per macro-tile
CHUNKS = G * RJ          # 8 chunks per macro
MACRO_ROWS = P * RJ * G  # 1024
CDIM = D + 2             # features + x_sq col + ones col
N_MACROS = 62
ROWS_PC = N_MACROS * MACRO_ROWS  # 63488 rows per core
N_PAD = ROWS_PC * N_CORES        # 507904

F32 = mybir.dt.float32

# normalize-op distribution per macro: (gpsimd, dve, act) chunks, sum == CHUNKS
NORM_SPLIT = (4, 4, 0)


def build_program(
    n_macros: int = N_MACROS,
    norm_split: tuple = NORM_SPLIT,
    passes: int = 1,
    stages: str = "full",  # dev probe: "dma" | "pe" | "dve" | "full"
) -> bass.Bass:
    rows = n_macros * MACRO_ROWS
    nc = bacc.Bacc()
    # x is pre-augmented on the host: 64 features + x_sq col + ones col
    x = nc.declare_dram_parameter("x", [rows, CDIM], F32, isOutput=False)
    w = nc.declare_dram_parameter("w", [CDIM, K], F32, isOutput=False)
    q = nc.declare_dram_parameter("q", [rows, K], F32, isOutput=True)
    ident = nc.inline_tensor(np.eye(P, dtype=np.float32), name="ident")

    # row(m, g, p, j) = m*1024 + g*256 + 2p + j
    x_v = x[:, :].rearrange("(m g p j) c -> m p g (j c)", g=G, p=P, j=RJ)
    q_v = q[:, :].rearrange("(m g p j) k -> m p g j k", g=G, p=P, j=RJ)

    with TileContext(nc) as tc, ExitStack() as ctx:
        consts = ctx.enter_context(tc.tile_pool(name="consts", bufs=1))
        w_t = consts.tile([CDIM, K], F32)
        nc.sync.dma_start(out=w_t[:, :], in_=w[:, :])
        id_t = consts.tile([P, P], F32)
        nc.sync.dma_start(out=id_t[:, :], in_=ident[:, :])
        qt_const = None
        if stages in ("dma", "pe"):
            qt_const = consts.tile([P, CHUNKS * K], F32)
            nc.vector.memset(qt_const[:, :], 0.5)

        xe_pool = ctx.enter_context(tc.tile_pool(name="xe", bufs=5))
        pt_pool = ctx.enter_context(tc.tile_pool(name="pt", bufs=2, space="PSUM"))
        xt_pool = ctx.enter_context(tc.tile_pool(name="xt", bufs=3))
        pa_pool = ctx.enter_context(tc.tile_pool(name="pa", bufs=2, space="PSUM"))
        qun_pool = ctx.enter_context(tc.tile_pool(name="qun", bufs=4))
        s_pool = ctx.enter_context(tc.tile_pool(name="s", bufs=6))
        r_pool = ctx.enter_context(tc.tile_pool(name="r", bufs=6))
        qt_pool = ctx.enter_context(tc.tile_pool(name="qt", bufs=5))

        def macro_body():
            for m in range(n_macros):
                emit_macro(m)

        def emit_macro(m):
            xe = xe_pool.tile([P, G * RJ * CDIM], F32)
            xe_v = xe[:, :].rearrange("p (g j c) -> p g j c", g=G, j=RJ)
            nc.sync.dma_start(out=xe[:, :], in_=x_v[m])
            if stages == "dma":
                qc_v = qt_const[:, :].rearrange("p (g j k) -> p g j k", g=G, j=RJ)
                nc.scalar.dma_start(out=q_v[m], in_=qc_v)
                return

            pt = pt_pool.tile([CDIM, CHUNKS * P], F32)  # 2 PSUM banks
            for c in range(CHUNKS):
                g, j = c // 2, c % 2
                nc.tensor.transpose(
                    pt[:, c * P : (c + 1) * P], xe_v[:, g, j, :], id_t[:, :]
                )
            xt = xt_pool.tile([CDIM, CHUNKS * P], F32)
            nc.scalar.copy(xt[:, :], pt[:, :])

            pa = pa_pool.tile([P, CHUNKS * P], F32)  # [128, 1024] = 2 banks
            for c in range(CHUNKS):
                nc.tensor.matmul(
                    pa[:, c * P : c * P + K],
                    xt[:, c * P : (c + 1) * P],
                    w_t[:, :],
                    start=True,
                    stop=True,
                )

            if stages == "pe":
                qc_v = qt_const[:, :].rearrange("p (g j k) -> p g j k", g=G, j=RJ)
                nc.scalar.dma_start(out=q_v[m], in_=qc_v)
                return

            qun = qun_pool.tile([P, CHUNKS * K], F32)
            pa_v = pa[:, :].rearrange("p (c s) -> p c s", c=CHUNKS)
            qun_v = qun[:, :].rearrange("p (c k) -> p c k", c=CHUNKS)
            nc.vector.reciprocal_approx_fast(out=qun_v, in_=pa_v[:, :, 0:K])

            n_gp, n_dve, n_act = norm_split
            assert n_gp + n_dve + n_act == CHUNKS
            if stages == "dve":
                s_t = s_pool.tile([P, CHUNKS], F32)
                nc.vector.tensor_reduce(
                    s_t[:, :], qun_v, axis=mybir.AxisListType.X, op=mybir.AluOpType.add
                )
                qu_v = qun[:, :].rearrange("p (g j k) -> p g j k", g=G, j=RJ)
                nc.scalar.dma_start(out=q_v[m], in_=qu_v)
                return

            qt = qt_pool.tile([P, CHUNKS * K], F32)
            # separate row-sum tiles per consumer engine: normalize_recip
            # writes 1/denom back in place, which must not WAR-chain with
            # the DVE reciprocal of the other chunks' sums
            if n_gp:
                s_gp = s_pool.tile([P, n_gp], F32)
                nc.vector.tensor_reduce(
                    s_gp[:, :],
                    qun_v[:, 0:n_gp, :],
                    axis=mybir.AxisListType.X,
                    op=mybir.AluOpType.add,
                )
            if n_dve or n_act:
                s_dr = s_pool.tile([P, n_dve + n_act], F32)
                nc.vector.tensor_reduce(
                    s_dr[:, :],
                    qun_v[:, n_gp:CHUNKS, :],
                    axis=mybir.AxisListType.X,
                    op=mybir.AluOpType.add,
                )
                r_t = r_pool.tile([P, n_dve + n_act], F32)
                nc.vector.reciprocal_approx_fast(out=r_t[:, :], in_=s_dr[:, :])
            for c in range(n_gp):
                nc.gpsimd.normalize_recip(
                    qt[:, c * K : (c + 1) * K],
                    qun[:, c * K : (c + 1) * K],
                    s_gp[:, c : c + 1],
                )
            for i in range(n_dve):
                c = n_gp + i
                nc.vector.tensor_scalar_mul(
                    qt[:, c * K : (c + 1) * K],
                    qun[:, c * K : (c + 1) * K],
                    r_t[:, i : i + 1],
                )
            for i in range(n_act):
                c = n_gp + n_dve + i
                nc.scalar.activation(
                    qt[:, c * K : (c + 1) * K],
                    qun[:, c * K : (c + 1) * K],
                    mybir.ActivationFunctionType.Copy,
                    scale=r_t[:, n_dve + i : n_dve + i + 1],
                )

            qt_v = qt[:, :].rearrange("p (g j k) -> p g j k", g=G, j=RJ)
            nc.scalar.dma_start(out=q_v[m], in_=qt_v)

        if passes > 1:
            with tc.For_i(0, passes, 1):
                macro_body()
        else:
            macro_body()

    nc.compile()
    return nc


def host_prep(x: np.ndarray, clusters: np.ndarray, rows_total: int):
    """Pad + build the augmented input [rows, 66] and centroid matrix."""
    n = x.shape[0]
    xall = np.zeros((rows_total, CDIM), dtype=np.float32)
    xall[:n, :D] = x
    xall[:n, D] = np.einsum("nd,nd->n", x, x)
    xall[:, D + 1] = 1.0
    c = np.asarray(clusters, dtype=np.float32)
    c_sq = np.einsum("kd,kd->k", c, c)
    w = np.empty((CDIM, K), dtype=np.float32)
    w[:D] = -2.0 * c.T
    w[D] = 1.0             # coefficient of the x_sq row
    w[D + 1] = 1.0 + c_sq  # coefficient of the ones row
    return xall, w


_CACHE: dict = {}

LAST_RESULT = None  # BassKernelResults of the most recent kernel() call


def kernel(inputs: np.ndarray, clusters: np.ndarray) -> np.ndarray:
    global LAST_RESULT
    x = np.ascontiguousarray(np.asarray(inputs, dtype=np.float32))
    n = x.shape[0]
    assert n == N and x.shape[1] == D, f"unexpected input shape {x.shape}"

    xall, w = host_prep(x, clusters, N_PAD)

    if "nc" not in _CACHE:
        _CACHE["nc"] = build_program()
    nc = _CACHE["nc"]

    in_maps = []
    for i in range(N_CORES):
        sl = slice(i * ROWS_PC, (i + 1) * ROWS_PC)
        in_maps.append({"x": np.ascontiguousarray(xall[sl]), "w": w})

    res = run_bass_kernel_spmd(nc, in_maps, list(range(N_CORES)))
    LAST_RESULT = res
    out = np.concatenate([res.results[i]["q"] for i in range(N_CORES)], axis=0)
    return np.ascontiguousarray(out[:n])


# revision 28
# speedup vs baseline: 1.4746x; 1.1220x over previous
"""Trainium2 Bass kernel: VQ-codebook soft assignment (ClusteringLayer).

q[n, k] = t_nk / sum_j t_nj,  t_nk = 1 / (1 + ||x_n - c_k||^2)
(reference has ALPHA = 1.0, so the power (ALPHA+1)/2 == 1.0 is the identity)

Strategy (data-parallel over 8 cores, rows sharded, centroids replicated):
 - host: pad N to 8*63488, precompute per-row x_sq, and the augmented
   centroid matrix W[66, 100] = [-2*C^T ; ones-coeff rows] so that ONE
   matmul per 128-row chunk produces a = 1 + ||x-c||^2 directly.
 - device, per 1024-row macro-tile:
     DMA in (rows packed 2/partition -> 512B contiguous reads)
     PE   : transpose [128, 66] chunks -> X^T (with x_sq + ones rows)
     PE   : matmul  X^T_aug (lhsT) @ W -> PSUM a = 1 + d2   [128, 100] x8
     DVE  : reciprocal_approx_fast (PSUM->SBUF)  t = 1/a    [128, 8, 100]
     DVE  : tensor_reduce row-sums s [128, 8]
     GPSIMD/ACT : normalize  q = t / s
     DMA out (800B contiguous writes)
"""

import os
from contextlib import ExitStack

import numpy as np

try:  # persistent XLA compile cache: makes repeat runs skip the walrus compile
    import jax

    jax.config.update("jax_compilation_cache_dir", "/tmp/jax_comp_cache")
    jax.config.update("jax_persistent_cache_min_entry_size_bytes", -1)
    jax.config.update("jax_persistent_cache_min_compile_time_secs", 0.0)
except Exception:
    pass

import concourse.bacc as bacc
import concourse.bass as bass
import concourse.mybir as mybir
from concourse.bass_utils import run_bass_kernel_spmd
from concourse.tile import TileContext

# problem constants (hardcoded; kernel.py must be self-contained)
N = 500_000
D = 64
K = 100
N_CORES = 8

P = 128                  # partitions; rows per chunk
RJ = 2                   # rows packed per partition
G = 4                    # row groups per macro-tile
CHUNKS = G * RJ          # 8 chunks per macro
MACRO_ROWS = P * RJ * G  # 1024
CDIM = D + 2             # features + x_sq col + ones col
N_MACROS = 62
ROWS_PC = N_MACROS * MACRO_ROWS  # 63488 rows per core
N_PAD = ROWS_PC * N_CORES        # 507904

F32 = mybir.dt.float32

# normalize-op distribution per macro: (gpsimd, dve, act) chunks, sum == CHUNKS
NORM_SPLIT = (4, 4, 0)


def build_program(
    n_macros: int = N_MACROS,
    norm_split: tuple = NORM_SPLIT,
    passes: int = 1,
    stages: str = "full",  # dev probe: "dma" | "pe" | "dve" | "full"
) -> bass.Bass:
    rows = n_macros * MACRO_ROWS
    nc = bacc.Bacc()
    # x is pre-augmented on the host: 64 features + x_sq col + ones col
    x = nc.declare_dram_parameter("x", [rows, CDIM], F32, isOutput=False)
    w = nc.declare_dram_parameter("w", [CDIM, K], F32, isOutput=False)
    q = nc.declare_dram_parameter("q", [rows, K], F32, isOutput=True)
    ident = nc.inline_tensor(np.eye(P, dtype=np.float32), name="ident")

    # row(m, g, p, j) = m*1024 + g*256 + 2p + j
    x_v = x[:, :].rearrange("(m g p j) c -> m p g (j c)", g=G, p=P, j=RJ)
    q_v = q[:, :].rearrange("(m g p j) k -> m p g j k", g=G, p=P, j=RJ)

    with TileContext(nc) as tc, ExitStack() as ctx:
        consts = ctx.enter_context(tc.tile_pool(name="consts", bufs=1))
        w_t = consts.tile([CDIM, K], F32)
        nc.sync.dma_start(out=w_t[:, :], in_=w[:, :])
        id_t = consts.tile([P, P], F32)
        nc.sync.dma_start(out=id_t[:, :], in_=ident[:, :])
        qt_const = None
        if stages in ("dma", "pe"):
            qt_const = consts.tile([P, CHUNKS * K], F32)
            nc.vector.memset(qt_const[:, :], 0.5)

        xe_pool = ctx.enter_context(tc.tile_pool(name="xe", bufs=5))
        pt_pool = ctx.enter_context(tc.tile_pool(name="pt", bufs=4, space="PSUM"))
        xt_pool = ctx.enter_context(tc.tile_pool(name="xt", bufs=6))
        pa_pool = ctx.enter_context(tc.tile_pool(name="pa", bufs=2, space="PSUM"))
        qun_pool = ctx.enter_context(tc.tile_pool(name="qun", bufs=4))
        s_pool = ctx.enter_context(tc.tile_pool(name="s", bufs=6))
        r_pool = ctx.enter_context(tc.tile_pool(name="r", bufs=6))
        qt_pool = ctx.enter_context(tc.tile_pool(name="qt", bufs=5))

        def macro_body():
            for m in range(n_macros):
                emit_macro(m)

        def emit_macro(m):
            xe = xe_pool.tile([P, G * RJ * CDIM], F32)
            xe_v = xe[:, :].rearrange("p (g j c) -> p g j c", g=G, j=RJ)
            nc.sync.dma_start(out=xe[:, :], in_=x_v[m])
            if stages == "dma":
                qc_v = qt_const[:, :].rearrange("p (g j k) -> p g j k", g=G, j=RJ)
                nc.scalar.dma_start(out=q_v[m], in_=qc_v)
                return

            xts = []
            for h in range(2):
                pt = pt_pool.tile([CDIM, 4 * P], F32)
                for c4 in range(4):
                    g = h * 2 + c4 // 2
                    j = c4 % 2
                    nc.tensor.transpose(
                        pt[:, c4 * P : (c4 + 1) * P], xe_v[:, g, j, :], id_t[:, :]
                    )
                xt = xt_pool.tile([CDIM, 4 * P], F32)
                nc.scalar.copy(xt[:, :], pt[:, :])
                xts.append(xt)

            pa = pa_pool.tile([P, CHUNKS * P], F32)  # [128, 1024] = 2 banks
            for c in range(CHUNKS):
                nc.tensor.matmul(
                    pa[:, c * P : c * P + K],
                    xts[c // 4][:, (c % 4) * P : (c % 4 + 1) * P],
                    w_t[:, :],
                    start=True,
                    stop=True,
                )

            if stages == "pe":
                qc_v = qt_const[:, :].rearrange("p (g j k) -> p g j k", g=G, j=RJ)
                nc.scalar.dma_start(out=q_v[m], in_=qc_v)
                return

            qun = qun_pool.tile([P, CHUNKS * K], F32)
            pa_v = pa[:, :].rearrange("p (c s) -> p c s", c=CHUNKS)
            qun_v = qun[:, :].rearrange("p (c k) -> p c k", c=CHUNKS)
            nc.vector.reciprocal_approx_fast(out=qun_v, in_=pa_v[:, :, 0:K])

            n_gp, n_dve, n_act = norm_split
            assert n_gp + n_dve + n_act == CHUNKS
            if stages == "dve":
                s_t = s_pool.tile([P, CHUNKS], F32)
                nc.vector.tensor_reduce(
                    s_t[:, :], qun_v, axis=mybir.AxisListType.X, op=mybir.AluOpType.add
                )
                qu_v = qun[:, :].rearrange("p (g j k) -> p g j k", g=G, j=RJ)
                nc.scalar.dma_start(out=q_v[m], in_=qu_v)
                return

            qt = qt_pool.tile([P, CHUNKS * K], F32)
            # separate row-sum tiles per consumer engine: normalize_recip
            # writes 1/denom back in place, which must not WAR-chain with
            # the DVE reciprocal of the other chunks' sums
            if n_gp:
                s_gp = s_pool.tile([P, n_gp], F32)
                nc.vector.tensor_reduce(
                    s_gp[:, :],
                    qun_v[:, 0:n_gp, :],
                    axis=mybir.AxisListType.X,
                    op=mybir.AluOpType.add,
                )
            if n_dve or n_act:
                s_dr = s_pool.tile([P, n_dve + n_act], F32)
                nc.vector.tensor_reduce(
                    s_dr[:, :],
                    qun_v[:, n_gp:CHUNKS, :],
                    axis=mybir.AxisListType.X,
                    op=mybir.AluOpType.add,
                )
                r_t = r_pool.tile([P, n_dve + n_act], F32)
                nc.vector.reciprocal_approx_fast(out=r_t[:, :], in_=s_dr[:, :])
            for c in range(n_gp):
                nc.gpsimd.normalize_recip(
                    qt[:, c * K : (c + 1) * K],
                    qun[:, c * K : (c + 1) * K],
                    s_gp[:, c : c + 1],
                )
            for i in range(n_dve):
                c = n_gp + i
                nc.vector.tensor_scalar_mul(
                    qt[:, c * K : (c + 1) * K],
                    qun[:, c * K : (c + 1) * K],
                    r_t[:, i : i + 1],
                )
            for i in range(n_act):
                c = n_gp + n_dve + i
                nc.scalar.activation(
                    qt[:, c * K : (c + 1) * K],
                    qun[:, c * K : (c + 1) * K],
                    mybir.ActivationFunctionType.Copy,
                    scale=r_t[:, n_dve + i : n_dve + i + 1],
                )

            qt_v = qt[:, :].rearrange("p (g j k) -> p g j k", g=G, j=RJ)
            nc.scalar.dma_start(out=q_v[m], in_=qt_v)

        if passes > 1:
            with tc.For_i(0, passes, 1):
                macro_body()
        else:
            macro_body()

    nc.compile()
    return nc


def host_prep(x: np.ndarray, clusters: np.ndarray, rows_total: int):
    """Pad + build the augmented input [rows, 66] and centroid matrix."""
    n = x.shape[0]
    xall = np.zeros((rows_total, CDIM), dtype=np.float32)
    xall[:n, :D] = x
    xall[:n, D] = np.einsum("nd,nd->n", x, x)
    xall[:, D + 1] = 1.0
    c = np.asarray(clusters, dtype=np.float32)
    c_sq = np.einsum("kd,kd->k", c, c)
    w = np.empty((CDIM, K), dtype=np.float32)
    w[:D] = -2.0 * c.T
    w[D] = 1.0             # coefficient of the x_sq row
    w[D + 1] = 1.0 + c_sq  # coefficient of the ones row
    return xall, w


_CACHE: dict = {}

LAST_RESULT = None  # BassKernelResults of the most recent kernel() call


def kernel(inputs: np.ndarray, clusters: np.ndarray) -> np.ndarray:
    global LAST_RESULT
    x = np.ascontiguousarray(np.asarray(inputs, dtype=np.float32))
    n = x.shape[0]
    assert n == N and x.shape[1] == D, f"unexpected input shape {x.shape}"

    xall, w = host_prep(x, clusters, N_PAD)

    if "nc" not in _CACHE:
        _CACHE["nc"] = build_program()
    nc = _CACHE["nc"]

    in_maps = []
    for i in range(N_CORES):
        sl = slice(i * ROWS_PC, (i + 1) * ROWS_PC)
        in_maps.append({"x": np.ascontiguousarray(xall[sl]), "w": w})

    res = run_bass_kernel_spmd(nc, in_maps, list(range(N_CORES)))
    LAST_RESULT = res
    out = np.concatenate([res.results[i]["q"] for i in range(N_CORES)], axis=0)
    return np.ascontiguousarray(out[:n])
